# revision 17
# baseline (speedup 1.0000x reference)
"""Self-contained Trainium2 Bass kernel for nn_LunarCausalAttention.

Sharding: 8 cores = 2 batches x 4 head-blocks (4 heads each). Params sliced
per core host-side; per-core partial outputs (over head-blocks) summed on
host during the gather (plus bo). Output is bf16 on device, f32 on host.

Restructured vs baseline:
- chunk scan is software-pipelined 4 deep so the PE never waits on the
  softmax vector/scalar chain (keeps the HAM clock warm at 2.4 GHz)
- per-chunk prefix states are snapshotted (P1b/P2b) so chunks only couple
  through cheap vector adds, not through the PE critical path
- tiny-element DMAs merged into one [128, 64] f32 "smallp" tensor; weights
  loaded as single large DMAs; xT split into 8 k-slices triggered on the
  Scalar queue so matmuls start as slices land
- head pairs run concurrently in the PE array via row/col tile_position
- softmax max-subtraction dropped (|logits| <= ~14), exp scale folds the
  1/((n+1)beta) length normalization via per-partition AP scale
"""

import math

import ml_dtypes
import numpy as np

import concourse.bacc as bacc
import concourse.bass as bass
import concourse.mybir as mybir
import concourse.tile as tile

EMBED = 1024
D = 64
PLEN = 32
NTOK = 2048
BSZ = 2
SCALING = D ** -0.5
BETA = math.log(2.0)

NH = 4           # heads per core
C = 128          # chunk (token tile)
NCH = NTOK // C  # 16 chunks
F32 = mybir.dt.float32
BF16 = mybir.dt.bfloat16
AX = mybir.AxisListType
AF = mybir.ActivationFunctionType

# smallp column map
SP_BQC = 0    # 4 cols
SP_BPQ = 4    # 2 cols
SP_BPC = 8    # 4 cols (bpc0[d, h], partitions 0-63)
SP_RLEN = 16  # 16 cols


def _bcast(ap_obj, dim_count, at=1):
    """Insert a stride-0 dim of size dim_count into an AP at free position."""
    pat = [list(p) for p in ap_obj.ap]
    pat.insert(at, [0, dim_count])
    return bass.AP(tensor=ap_obj.tensor, offset=ap_obj.offset, ap=pat)


def build_nc():
    nc = bacc.Bacc("TRN2", target_bir_lowering=False, debug=False,
                   num_devices=8)

    xT_d = nc.dram_tensor("xT", [EMBED, NTOK], BF16, kind="ExternalInput")
    pxT_d = nc.dram_tensor("pxT", [EMBED, PLEN], BF16, kind="ExternalInput")
    wqc_d = nc.dram_tensor("wqcT", [EMBED, 4 * C], BF16, kind="ExternalInput")
    wpq_d = nc.dram_tensor("wpqT", [EMBED, 2 * C], BF16, kind="ExternalInput")
    wpc_d = nc.dram_tensor("wpcR", [D, NH, 8, C], BF16, kind="ExternalInput")
    wo_d = nc.dram_tensor("woT", [NH * D, EMBED], BF16, kind="ExternalInput")
    smallp_d = nc.dram_tensor("smallp", [C, 64], F32, kind="ExternalInput")
    mask_d = nc.dram_tensor("mask", [C, C], F32, kind="ExternalInput")
    id128_d = nc.dram_tensor("id128", [C, C], BF16, kind="ExternalInput")
    out_d = nc.dram_tensor("out", [NTOK, EMBED], BF16, kind="ExternalOutput")

    with tile.TileContext(nc) as tc:
        with (
            tc.tile_pool(name="big", bufs=1) as big,
            tc.tile_pool(name="work", bufs=2) as work,
            tc.tile_pool(name="outp", bufs=2) as outp,
            tc.tile_pool(name="psp", bufs=1, space="PSUM") as psp,
        ):
            # ---- persistent tiles ----
            pxT = big.tile([128, 8, PLEN], BF16)
            smallp = big.tile([128, 64], F32)
            wpq = big.tile([128, 8, 2 * C], BF16)
            xT = big.tile([128, 8, NTOK], BF16)
            wpc = big.tile([D, NH, 8, C], BF16)
            wqc = big.tile([128, 8, 4 * C], BF16)
            wo = big.tile([128, 2, EMBED], BF16)
            mask = big.tile([C, C], F32)
            id128 = big.tile([C, C], BF16)

            # ---- DMA triggers: critical path first; xT on scalar queue ----
            nc.sync.dma_start(out=pxT,
                              in_=pxT_d.rearrange("(k p) n -> p k n", p=128))
            nc.sync.dma_start(out=smallp, in_=smallp_d.ap())
            nc.sync.dma_start(out=wpq,
                              in_=wpq_d.rearrange("(k p) m -> p k m", p=128))
            xT_r = xT_d.rearrange("(k p) n -> p k n", p=128)
            for k in range(8):
                nc.gpsimd.dma_start(out=xT[:, k, :], in_=xT_r[:, k, :])
            nc.sync.dma_start(out=wqc,
                              in_=wqc_d.rearrange("(k p) m -> p k m", p=128))
            nc.sync.dma_start(out=wpc, in_=wpc_d.ap())
            nc.sync.dma_start(out=mask, in_=mask_d.ap())
            nc.sync.dma_start(out=id128, in_=id128_d.ap())
            nc.sync.dma_start(out=wo,
                              in_=wo_d.rearrange("(k p) o -> p k o", p=128))

            # ---- more persistents (SBUF state) ----
            lin = big.tile([128, 4, NTOK], BF16)      # q(0,1) kv(2,3)
            z_cm = big.tile([128, NTOK], BF16)        # z channel-major (h,p)
            z_tok = big.tile([128, NCH, C], BF16)     # z token-major
            kvtok = big.tile([128, NCH, 2, 2, D], BF16)  # (c, g, half, d)
            weff = big.tile([128, 8, C], BF16)
            pq_sb = big.tile([128, 2, PLEN], BF16)
            pq0 = big.tile([D, 2, PLEN], BF16)        # odd halves at base 0
            lin0 = big.tile([D, 4, NTOK], BF16)       # odd halves at base 0
            z_cm2 = big.tile([32, 3, NTOK], BF16)     # z strips h=1,2,3 base 0
            bpc0 = big.tile([D, NH], BF16)
            be_beta = big.tile([128, 1], F32)
            P1b = big.tile([D, NCH, NH, PLEN], BF16)
            P2b = big.tile([32, NCH, NH, D], BF16)
            awT2 = big.tile([32, NCH, NH, C], BF16)   # aw^T strips base 0

            nc.vector.tensor_copy(bpc0, smallp[0:D, SP_BPC:SP_BPC + 4])

            def q_at0(h):
                g, half = h // 2, h % 2
                return lin0[:, g, :] if half else lin[0:D, g, :]

            def kv_at0(h):
                g, half = h // 2, h % 2
                return lin0[:, 2 + g, :] if half else lin[0:D, 2 + g, :]

            def pq_at0(h):
                g, half = h // 2, h % 2
                return pq0[:, g, :] if half else pq_sb[0:D, g, :]

            def z_strip(h):
                return z_cm[0:32, :] if h == 0 else z_cm2[:, h - 1, :]

            # ---- pq linear: pq_sb[64h+d, g, p], scaled (host prescaled) ----
            for m in range(2):
                ps = psp.tile([128, PLEN], F32, tag="pD", name="pq_ps")
                for k in range(8):
                    nc.tensor.matmul(ps, lhsT=wpq[:, k, m * 128:(m + 1) * 128],
                                     rhs=pxT[:, k, :],
                                     start=(k == 0), stop=(k == 7))
                nc.scalar.activation(out=pq_sb[:, m, :], in_=ps,
                                     func=AF.Identity,
                                     bias=smallp[:, SP_BPQ + m:SP_BPQ + m + 1],
                                     scale=1.0)

            for g in range(2):
                nc.sync.dma_start(out=pq0[:, g, :], in_=pq_sb[D:128, g, :])

            # ---- q/kv linears (4 psum banks; weights persistent) ----
            for m in range(4):
                pss = [psp.tile([128, 512], F32, tag=t, name=f"lin_{t}")
                       for t in ("pA", "pB", "pC", "pD")]
                for k in range(8):
                    for nt in range(4):
                        nc.tensor.matmul(pss[nt],
                                         lhsT=wqc[:, k, m * 128:(m + 1) * 128],
                                         rhs=xT[:, k, nt * 512:(nt + 1) * 512],
                                         start=(k == 0), stop=(k == 7))
                for nt in range(4):
                    nc.scalar.activation(
                        out=lin[:, m, nt * 512:(nt + 1) * 512], in_=pss[nt],
                        func=AF.Identity,
                        bias=smallp[:, SP_BQC + m:SP_BQC + m + 1], scale=1.0)

            for j in range(4):
                nc.sync.dma_start(out=lin0[:, j, :], in_=lin[D:128, j, :])

            # ---- W_eff[e,(h,p)] = sum_d Wpc[(h,d),e] * pq[h,p,d] ----
            for k in range(8):
                ps = psp.tile([128, NH, PLEN], F32,
                              tag=("pA" if k % 2 == 0 else "pB"),
                              name=f"weff_ps{k % 2}")
                for h in range(NH):
                    nc.tensor.matmul(ps[:, h, :], lhsT=wpc[:, h, k, :],
                                     rhs=pq_at0(h),
                                     start=True, stop=True)
                nc.scalar.copy(weff[:, k, :],
                               ps.rearrange("p h w -> p (h w)"))

            # ---- be_beta[(h,p)] = BETA * sum_d bpc[h,d] pq[h,p,d] ----
            be_ps = psp.tile([128, 1], F32, tag="pG", name="be_ps")
            for h in range(NH):
                nc.tensor.matmul(be_ps[32 * h:32 * h + 32, :],
                                 lhsT=pq_at0(h),
                                 rhs=bpc0[:, h:h + 1],
                                 start=True, stop=True,
                                 tile_position=(0, 32 * h))
            nc.scalar.mul(be_beta, be_ps, BETA)

            # ---- pattn channel-major + z = ln(1+exp(beta*pattn+beta*be)) ----
            for nt in range(4):
                sl = slice(nt * 512, (nt + 1) * 512)
                pps = psp.tile([128, 512], F32, tag=("pE" if nt % 2 == 0
                                                     else "pD"),
                               name=f"pat{nt % 2}")
                for k in range(8):
                    nc.tensor.matmul(pps, lhsT=weff[:, k, :],
                                     rhs=xT[:, k, sl],
                                     start=(k == 0), stop=(k == 7))
                nc.scalar.activation(out=z_cm[:, sl], in_=pps, func=AF.Exp,
                                     scale=BETA, bias=be_beta)
            nc.scalar.activation(out=z_cm, in_=z_cm, func=AF.Ln, bias=1.0)
            for h in range(1, NH):
                nc.sync.dma_start(out=z_cm2[:, h - 1, :],
                                  in_=z_cm[32 * h:32 * h + 32, :])

            # ---- kv transposes to token-major ----
            for c in range(NCH):
                tok = slice(c * C, (c + 1) * C)
                for g in range(2):
                    j = 2 * c + g
                    ps = psp.tile([128, 128], F32,
                                  tag=("pF" if j % 2 == 0 else "pE"),
                                  name=f"kvt{j % 2}")
                    nc.tensor.matmul(ps, lhsT=lin[:, 2 + g, tok], rhs=id128,
                                     start=True, stop=True)
                    dst = kvtok[:, c, g].rearrange("p h d -> p (h d)")
                    if g == 0:
                        nc.vector.tensor_copy(dst, ps)
                    else:
                        nc.scalar.copy(dst, ps)

            # ---- software-pipelined chunk scan ----
            # slots per iteration i: A=i (m1/ztp/dS), B=i-1 (o1/softmax/awT),
            # C2=i-2 (m2/out2), PJ=i-3 (proj)
            m1m_t = [None] * NCH
            m2m_t = [None] * NCH
            aw_t = [None] * NCH
            attnT_t = [None] * NCH
            fp_t = [None] * NCH
            osb_t = [None] * (NCH // 4)

            # persistent PSUM prefix-state accumulator:
            # dS1 prefix in [:, 0:128] ([64d, 4h, 32p]),
            # dS2 prefix in [0:32, 128:384] ([32p, 4h, 64d])
            ps_state = psp.tile([D, 384], F32, tag="pG", name="ps_state")
            sP1 = ps_state[:, 0:128].rearrange("p (h w) -> p h w", w=PLEN)
            sP2 = ps_state[0:32, 128:384].rearrange("p (h w) -> p h w", w=D)
            zr = big.tile([1, 384], BF16)
            nc.vector.memset(zr, 0.0)
            # prime the accumulator: zero-write the whole region once so all
            # chunk contributions can accumulate with start=False
            nc.tensor.matmul(ps_state[:, :], lhsT=zr[:, 0:D], rhs=zr,
                             start=True, stop=False, skip_group_check=True)

            def head_slices(h):
                g, half = h // 2, h % 2
                return g, half, slice(64 * half, 64 * half + 64)

            for i in range(NCH + 4):
                a, b, c2, pj = i, i - 1, i - 2, i - 3

                # --- evac proj(e = i-4) psum -> out_sb; DMA per 4 chunks ---
                e = i - 4
                if 0 <= e < NCH:
                    fpe = fp_t[e]
                    osb = osb_t[e // 4]
                    nc.vector.tensor_copy(osb[:, e % 4, 0:512], fpe[:, 0, :])
                    nc.scalar.copy(osb[:, e % 4, 512:1024], fpe[:, 1, :])
                    fp_t[e] = None
                    if e % 4 == 3:
                        gidx = e // 4
                        dst = out_d.rearrange("(g c p) e -> g p c e",
                                              c=4, p=128)[gidx]
                        nc.sync.dma_start(out=dst, in_=osb)

                # --- PE: m2(c2) ---
                if 0 <= c2 < NCH:
                    tok2 = slice(c2 * C, (c2 + 1) * C)
                    m2_ps = psp.tile([128, NH, C], F32, tag="pB", name="m2")
                    for h in range(NH):
                        nc.tensor.matmul(m2_ps[:, h, :],
                                         lhsT=z_strip(h)[:, tok2],
                                         rhs=awT2[:, c2, h, :],
                                         start=True, stop=True)

                # --- PE: proj(pj); evac deferred to next iteration ---
                if 0 <= pj < NCH:
                    attnT = attnT_t[pj]
                    fp = psp.tile([128, 2, 512], F32, tag="pC",
                                  name="proj")
                    for nh in range(2):
                        osl = slice(nh * 512, (nh + 1) * 512)
                        for kt in range(2):
                            nc.tensor.matmul(fp[:, nh, :], lhsT=attnT[:, kt, :],
                                             rhs=wo[:, kt, osl],
                                             start=(kt == 0), stop=(kt == 1))
                    attnT_t[pj] = None
                    fp_t[pj] = fp

                # --- PE: ztp(a), m1(a) ---
                if a < NCH:
                    toka = slice(a * C, (a + 1) * C)
                    ztp = psp.tile([128, 128], F32, tag="pF", name="ztp")
                    nc.tensor.matmul(ztp, lhsT=z_cm[:, toka], rhs=id128,
                                     start=True, stop=True)
                    nc.vector.tensor_copy(z_tok[:, a, :], ztp)

                    m1_ps = psp.tile([128, NH, C], F32, tag="pA", name="m1")
                    for h in range(NH):
                        nc.tensor.matmul(m1_ps[:, h, :],
                                         lhsT=kv_at0(h)[:, toka],
                                         rhs=q_at0(h)[:, toka],
                                         start=True, stop=True)
                    m1m = work.tile([128, NH, C], BF16, tag="m1m")
                    nc.vector.tensor_mul(m1m, m1_ps, _bcast(mask, NH))
                    m1m_t[a] = m1m

                # --- vector: m2m(c2) ---
                if 0 <= c2 < NCH:
                    m2m = work.tile([128, NH, C], BF16, tag="m2m")
                    nc.vector.tensor_mul(m2m, m2_ps, _bcast(mask, NH))
                    m2m_t[c2] = m2m

                # --- PE: o1(b) + ACT exp + V softmax ---
                if 0 <= b < NCH:
                    tokb = slice(b * C, (b + 1) * C)
                    o1 = psp.tile([128, NH, PLEN], F32, tag="pD", name="o1")
                    m1mb = m1m_t[b]
                    for h in range(NH):
                        nc.tensor.matmul(o1[:, h, :], lhsT=m1mb[:, h, :],
                                         rhs=z_tok[:, b, 32 * h:32 * h + 32],
                                         start=True, stop=(b == 0))
                        if b > 0:
                            nc.tensor.matmul(o1[:, h, :],
                                             lhsT=q_at0(h)[:, tokb],
                                             rhs=P1b[:, b, h, :],
                                             start=False, stop=True)
                    m1m_t[b] = None
                    rl = smallp[:, SP_RLEN + b:SP_RLEN + b + 1]
                    e_sb = work.tile([128, NH, PLEN], F32, tag="e_sb")
                    nc.scalar.activation(out=e_sb, in_=o1, func=AF.Exp,
                                         scale=rl, bias=0.0)
                    ssum = work.tile([128, NH], F32, tag="ssum")
                    nc.vector.reduce_sum(ssum, e_sb, axis=AX.X)
                    rs = work.tile([128, NH], F32, tag="rs")
                    nc.vector.reciprocal(rs, ssum)
                    rs2 = work.tile([128, NH], F32, tag="rs2")
                    nc.vector.tensor_scalar_mul(rs2, rs, rl)
                    aw = work.tile([128, NH, PLEN], BF16, tag="aw")
                    nc.vector.tensor_mul(aw, e_sb, _bcast(rs2, PLEN, at=2))
                    aw_t[b] = aw

                # --- PE: dS1(a) dS2(a); V: prefix snapshots + adds ---
                if a < NCH:
                    if a > 0:
                        nc.vector.tensor_copy(P1b[:, a], sP1)
                        nc.vector.tensor_copy(P2b[:, a], sP2)
                    for h in range(NH):
                        g, half, _ = head_slices(h)
                        nc.tensor.matmul(sP1[:, h, :],
                                         lhsT=kvtok[:, a, g, half, :],
                                         rhs=z_tok[:, a, 32 * h:32 * h + 32],
                                         start=False, stop=False,
                                         skip_group_check=True)
                    for h in range(NH):
                        g, half, _ = head_slices(h)
                        nc.tensor.matmul(sP2[:, h, :],
                                         lhsT=z_tok[:, a, 32 * h:32 * h + 32],
                                         rhs=kvtok[:, a, g, half, :],
                                         start=False,
                                         stop=(a == NCH - 1 and h == NH - 1),
                                         skip_group_check=True)

                # --- PE: out2(c2); ACT: attnT evac ---
                if 0 <= c2 < NCH:
                    attn_ps = psp.tile([128, 2, C], F32, tag="pE",
                                       name="attn")
                    m2mc = m2m_t[c2]
                    for h in range(NH):
                        g, half, s = head_slices(h)
                        nc.tensor.matmul(attn_ps[s, g, :],
                                         lhsT=kvtok[:, c2, g, half, :],
                                         rhs=m2mc[:, h, :],
                                         start=True, stop=(c2 == 0))
                        if c2 > 0:
                            nc.tensor.matmul(attn_ps[s, g, :],
                                             lhsT=P2b[:, c2, h, :],
                                             rhs=awT2[:, c2, h, :],
                                             start=False, stop=True,
                                             tile_position=(0, 64 * (h % 2)))
                    m2m_t[c2] = None
                    attnT = work.tile([128, 2, C], BF16, tag="attnT")
                    nc.scalar.copy(attnT, attn_ps)
                    attnT_t[c2] = attnT

                # --- PE: awT(b) strip transposes; ACT: evac ---
                if 0 <= b < NCH:
                    awp = psp.tile([32, NH, C], F32, tag="pF", name="awp")
                    awb = aw_t[b]
                    for h in range(NH):
                        nc.tensor.matmul(awp[:, h, :], lhsT=awb[:, h, :],
                                         rhs=id128, start=True, stop=True)
                    aw_t[b] = None
                    nc.scalar.copy(awT2[:, b], awp)

                # allocate out_sb tile at group start
                if a < NCH and a % 4 == 0:
                    osb_t[a // 4] = outp.tile([128, 4, EMBED], BF16,
                                              tag="osb", name="osb")

    nc.compile()
    return nc


_NC = None


def get_nc():
    global _NC
    if _NC is None:
        _NC = build_nc()
    return _NC


def make_in_maps(query, pquery, Wpq, bpq, Wq, bq, Wpc, bpc, Wc, bc, Wo, bo):
    query = np.asarray(query, np.float32)
    pquery = np.asarray(pquery, np.float32)
    Wpq, Wq, Wpc, Wc, Wo = (np.asarray(w, np.float32)
                            for w in (Wpq, Wq, Wpc, Wc, Wo))
    bpq_, bq_, bpc_, bc_ = (np.asarray(v, np.float32)
                            for v in (bpq, bq, bpc, bc))
    n_idx = np.arange(NTOK, dtype=np.float64)
    rlen = (1.0 / ((n_idx + 1.0) * BETA)).astype(np.float32)
    rlen = np.ascontiguousarray(rlen.reshape(NCH, C).T)          # [C, NCH]
    mask = np.triu(np.ones((C, C), np.float32))                  # keep j <= i
    id128 = np.eye(128, dtype=np.float32)
    bf = ml_dtypes.bfloat16

    in_maps = []
    for core in range(8):
        b, hb = core // 4, core % 4
        ch = slice(hb * NH * D, (hb + 1) * NH * D)
        wqcT = np.concatenate([SCALING * Wq[ch], Wc[ch]], axis=0).T
        bqc = np.concatenate([SCALING * bq_[ch], bc_[ch]])       # [512]
        bpqs = SCALING * bpq_[ch]                                # [256]
        bpcs = bpc_[ch]                                          # [256]
        # wpcR[d, h, k, e] = Wpc_core[h*64+d, 128k+e]
        wpcR = np.ascontiguousarray(
            Wpc[ch].reshape(NH, D, 8, 128).transpose(1, 0, 2, 3))
        smallp = np.zeros((128, 64), np.float32)
        smallp[:, SP_BQC:SP_BQC + 4] = bqc.reshape(4, 128).T
        smallp[:, SP_BPQ:SP_BPQ + 2] = bpqs.reshape(2, 128).T
        # bpc0[d, h] = bpc_core[h*64+d] on partitions 0-63
        smallp[0:D, SP_BPC:SP_BPC + 4] = bpcs.reshape(NH, D).T
        smallp[:, SP_RLEN:SP_RLEN + NCH] = rlen
        in_maps.append({
            "xT": np.ascontiguousarray(query[:, b, :].T).astype(bf),
            "pxT": np.ascontiguousarray(pquery[:, b, :].T).astype(bf),
            "wqcT": np.ascontiguousarray(wqcT).astype(bf),
            "wpqT": np.ascontiguousarray((SCALING * Wpq[ch]).T).astype(bf),
            "wpcR": wpcR.astype(bf),
            "woT": np.ascontiguousarray(Wo[:, ch].T).astype(bf),
            "smallp": smallp,
            "mask": mask,
            "id128": id128.astype(bf),
        })
    return in_maps


def kernel(**inputs):
    from concourse.bass_utils import run_bass_kernel_spmd
    nc = get_nc()
    in_maps = make_in_maps(**inputs)
    res = run_bass_kernel_spmd(nc, in_maps, core_ids=list(range(8)))
    bo = np.asarray(inputs["bo"], np.float32)
    out = np.zeros((NTOK, BSZ, EMBED), np.float32)
    for b in range(BSZ):
        acc = res.results[4 * b]["out"].astype(np.float32)
        for i in range(1, 4):
            acc = acc + res.results[4 * b + i]["out"].astype(np.float32)
        out[:, b, :] = acc + bo
    return out


# revision 18
# speedup vs baseline: 1.0232x; 1.0232x over previous
"""Self-contained Trainium2 Bass kernel for nn_LunarCausalAttention.

Sharding: 8 cores = 2 batches x 4 head-blocks (4 heads each). Params sliced
per core host-side; per-core partial outputs (over head-blocks) summed on
host during the gather (plus bo). Output is bf16 on device, f32 on host.

Restructured vs baseline:
- chunk scan is software-pipelined 4 deep so the PE never waits on the
  softmax vector/scalar chain (keeps the HAM clock warm at 2.4 GHz)
- per-chunk prefix states are snapshotted (P1b/P2b) so chunks only couple
  through cheap vector adds, not through the PE critical path
- tiny-element DMAs merged into one [128, 64] f32 "smallp" tensor; weights
  loaded as single large DMAs; xT split into 8 k-slices triggered on the
  Scalar queue so matmuls start as slices land
- head pairs run concurrently in the PE array via row/col tile_position
- softmax max-subtraction dropped (|logits| <= ~14), exp scale folds the
  1/((n+1)beta) length normalization via per-partition AP scale
"""

import math

import ml_dtypes
import numpy as np

import concourse.bacc as bacc
import concourse.bass as bass
import concourse.mybir as mybir
import concourse.tile as tile

EMBED = 1024
D = 64
PLEN = 32
NTOK = 2048
BSZ = 2
SCALING = D ** -0.5
BETA = math.log(2.0)

NH = 4           # heads per core
C = 128          # chunk (token tile)
NCH = NTOK // C  # 16 chunks
F32 = mybir.dt.float32
BF16 = mybir.dt.bfloat16
AX = mybir.AxisListType
AF = mybir.ActivationFunctionType

# smallp column map
SP_BQC = 0    # 4 cols
SP_BPQ = 4    # 2 cols
SP_BPC = 8    # 4 cols (bpc0[d, h], partitions 0-63)
SP_RLEN = 16  # 16 cols


def _bcast(ap_obj, dim_count, at=1):
    """Insert a stride-0 dim of size dim_count into an AP at free position."""
    pat = [list(p) for p in ap_obj.ap]
    pat.insert(at, [0, dim_count])
    return bass.AP(tensor=ap_obj.tensor, offset=ap_obj.offset, ap=pat)


def build_nc():
    nc = bacc.Bacc("TRN2", target_bir_lowering=False, debug=False,
                   num_devices=8)

    xT_d = nc.dram_tensor("xT", [EMBED, NTOK], BF16, kind="ExternalInput")
    pxT_d = nc.dram_tensor("pxT", [EMBED, PLEN], BF16, kind="ExternalInput")
    wqc_d = nc.dram_tensor("wqcT", [EMBED, 4 * C], BF16, kind="ExternalInput")
    wpq_d = nc.dram_tensor("wpqT", [EMBED, 2 * C], BF16, kind="ExternalInput")
    wpc_d = nc.dram_tensor("wpcR", [D, NH, 8, C], BF16, kind="ExternalInput")
    wo_d = nc.dram_tensor("woT", [NH * D, EMBED], BF16, kind="ExternalInput")
    smallp_d = nc.dram_tensor("smallp", [C, 64], F32, kind="ExternalInput")
    mask_d = nc.dram_tensor("mask", [C, C], F32, kind="ExternalInput")
    id128_d = nc.dram_tensor("id128", [C, C], BF16, kind="ExternalInput")
    out_d = nc.dram_tensor("out", [NTOK, EMBED], BF16, kind="ExternalOutput")

    with tile.TileContext(nc) as tc:
        with (
            tc.tile_pool(name="big", bufs=1) as big,
            tc.tile_pool(name="work", bufs=2) as work,
            tc.tile_pool(name="outp", bufs=2) as outp,
            tc.tile_pool(name="psp", bufs=1, space="PSUM") as psp,
        ):
            # ---- persistent tiles ----
            pxT = big.tile([128, 8, PLEN], BF16)
            smallp = big.tile([128, 64], F32)
            wpq = big.tile([128, 8, 2 * C], BF16)
            xT = big.tile([128, 8, NTOK], BF16)
            wpc = big.tile([D, NH, 8, C], BF16)
            wqc = big.tile([128, 8, 4 * C], BF16)
            wo = big.tile([128, 2, EMBED], BF16)
            mask = big.tile([C, C], F32)
            id128 = big.tile([C, C], BF16)

            # ---- DMA triggers: critical path first; xT on scalar queue ----
            nc.sync.dma_start(out=pxT,
                              in_=pxT_d.rearrange("(k p) n -> p k n", p=128))
            nc.sync.dma_start(out=smallp, in_=smallp_d.ap())
            nc.sync.dma_start(out=wpq,
                              in_=wpq_d.rearrange("(k p) m -> p k m", p=128))
            xT_r = xT_d.rearrange("(k p) n -> p k n", p=128)
            for k in range(8):
                nc.gpsimd.dma_start(out=xT[:, k, :], in_=xT_r[:, k, :])
            nc.sync.dma_start(out=wqc,
                              in_=wqc_d.rearrange("(k p) m -> p k m", p=128))
            nc.sync.dma_start(out=wpc, in_=wpc_d.ap())
            nc.sync.dma_start(out=mask, in_=mask_d.ap())
            nc.sync.dma_start(out=id128, in_=id128_d.ap())
            nc.sync.dma_start(out=wo,
                              in_=wo_d.rearrange("(k p) o -> p k o", p=128))

            # ---- more persistents (SBUF state) ----
            lin = big.tile([128, 4, NTOK], BF16)      # q(0,1) kv(2,3)
            z_cm = big.tile([128, NTOK], BF16)        # z channel-major (h,p)
            z_tok = big.tile([128, NCH, C], BF16)     # z token-major
            kvtok = big.tile([128, NCH, 2, 2, D], BF16)  # (c, g, half, d)
            weff = big.tile([128, 8, C], BF16)
            pq_sb = big.tile([128, 2, PLEN], BF16)
            pq0 = big.tile([D, 2, PLEN], BF16)        # odd halves at base 0
            lin0 = big.tile([D, 4, NTOK], BF16)       # odd halves at base 0
            z_cm2 = big.tile([32, 3, NTOK], BF16)     # z strips h=1,2,3 base 0
            bpc0 = big.tile([D, NH], BF16)
            be_beta = big.tile([128, 1], F32)
            P1b = big.tile([D, NCH, NH, PLEN], BF16)
            P2b = big.tile([32, NCH, NH, D], BF16)
            awT2 = big.tile([32, NCH, NH, C], BF16)   # aw^T strips base 0

            nc.vector.tensor_copy(bpc0, smallp[0:D, SP_BPC:SP_BPC + 4])

            def q_at0(h):
                g, half = h // 2, h % 2
                return lin0[:, g, :] if half else lin[0:D, g, :]

            def kv_at0(h):
                g, half = h // 2, h % 2
                return lin0[:, 2 + g, :] if half else lin[0:D, 2 + g, :]

            def pq_at0(h):
                g, half = h // 2, h % 2
                return pq0[:, g, :] if half else pq_sb[0:D, g, :]

            def z_strip(h):
                return z_cm[0:32, :] if h == 0 else z_cm2[:, h - 1, :]

            # ---- pq linear: pq_sb[64h+d, g, p], scaled (host prescaled) ----
            for m in range(2):
                ps = psp.tile([128, PLEN], F32, tag="pD", name="pq_ps")
                for k in range(8):
                    nc.tensor.matmul(ps, lhsT=wpq[:, k, m * 128:(m + 1) * 128],
                                     rhs=pxT[:, k, :],
                                     start=(k == 0), stop=(k == 7))
                nc.scalar.activation(out=pq_sb[:, m, :], in_=ps,
                                     func=AF.Identity,
                                     bias=smallp[:, SP_BPQ + m:SP_BPQ + m + 1],
                                     scale=1.0)

            for g in range(2):
                nc.sync.dma_start(out=pq0[:, g, :], in_=pq_sb[D:128, g, :])

            # ---- q/kv linears (4 psum banks; weights persistent) ----
            for m in range(4):
                pss = [psp.tile([128, 512], F32, tag=t, name=f"lin_{t}")
                       for t in ("pA", "pB", "pC", "pD")]
                for k in range(8):
                    for nt in range(4):
                        nc.tensor.matmul(pss[nt],
                                         lhsT=wqc[:, k, m * 128:(m + 1) * 128],
                                         rhs=xT[:, k, nt * 512:(nt + 1) * 512],
                                         start=(k == 0), stop=(k == 7))
                for nt in range(4):
                    nc.scalar.activation(
                        out=lin[:, m, nt * 512:(nt + 1) * 512], in_=pss[nt],
                        func=AF.Identity,
                        bias=smallp[:, SP_BQC + m:SP_BQC + m + 1], scale=1.0)

            for j in range(4):
                nc.sync.dma_start(out=lin0[:, j, :], in_=lin[D:128, j, :])

            # ---- W_eff[e,(h,p)] = sum_d Wpc[(h,d),e] * pq[h,p,d] ----
            for k in range(8):
                ps = psp.tile([128, NH, PLEN], F32,
                              tag=("pA" if k % 2 == 0 else "pB"),
                              name=f"weff_ps{k % 2}")
                for h in range(NH):
                    nc.tensor.matmul(ps[:, h, :], lhsT=wpc[:, h, k, :],
                                     rhs=pq_at0(h),
                                     start=True, stop=True)
                nc.scalar.copy(weff[:, k, :],
                               ps.rearrange("p h w -> p (h w)"))

            # ---- be_beta[(h,p)] = BETA * sum_d bpc[h,d] pq[h,p,d] ----
            be_ps = psp.tile([128, 1], F32, tag="pG", name="be_ps")
            for h in range(NH):
                nc.tensor.matmul(be_ps[32 * h:32 * h + 32, :],
                                 lhsT=pq_at0(h),
                                 rhs=bpc0[:, h:h + 1],
                                 start=True, stop=True,
                                 tile_position=(0, 32 * h))
            nc.scalar.mul(be_beta, be_ps, BETA)

            # ---- pattn channel-major + z = ln(1+exp(beta*pattn+beta*be)) ----
            for nt in range(4):
                sl = slice(nt * 512, (nt + 1) * 512)
                pps = psp.tile([128, 512], F32, tag=("pE" if nt % 2 == 0
                                                     else "pD"),
                               name=f"pat{nt % 2}")
                for k in range(8):
                    nc.tensor.matmul(pps, lhsT=weff[:, k, :],
                                     rhs=xT[:, k, sl],
                                     start=(k == 0), stop=(k == 7))
                nc.scalar.activation(out=z_cm[:, sl], in_=pps, func=AF.Exp,
                                     scale=BETA, bias=be_beta)
            nc.scalar.activation(out=z_cm, in_=z_cm, func=AF.Ln, bias=1.0)
            for h in range(1, NH):
                nc.sync.dma_start(out=z_cm2[:, h - 1, :],
                                  in_=z_cm[32 * h:32 * h + 32, :])

            # ---- kv transposes to token-major ----
            for c in range(NCH):
                tok = slice(c * C, (c + 1) * C)
                for g in range(2):
                    j = 2 * c + g
                    ps = psp.tile([128, 128], F32,
                                  tag=("pF" if j % 2 == 0 else "pE"),
                                  name=f"kvt{j % 2}")
                    nc.tensor.matmul(ps, lhsT=lin[:, 2 + g, tok], rhs=id128,
                                     start=True, stop=True)
                    dst = kvtok[:, c, g].rearrange("p h d -> p (h d)")
                    if g == 0:
                        nc.vector.tensor_copy(dst, ps)
                    else:
                        nc.scalar.copy(dst, ps)

            # ---- software-pipelined chunk scan ----
            # slots per iteration i: A=i (m1/ztp/dS), B=i-1 (o1/softmax/awT),
            # C2=i-2 (m2/out2), PJ=i-3 (proj)
            m1m_t = [None] * NCH
            m2m_t = [None] * NCH
            aw_t = [None] * NCH
            attnT_t = [None] * NCH
            fp_t = [None] * NCH
            osb_t = [None] * (NCH // 4)

            # persistent PSUM prefix-state accumulator:
            # dS1 prefix in [:, 0:128] ([64d, 4h, 32p]),
            # dS2 prefix in [0:32, 128:384] ([32p, 4h, 64d])
            ps_state = psp.tile([D, 384], F32, tag="pG", name="ps_state")
            sP1 = ps_state[:, 0:128].rearrange("p (h w) -> p h w", w=PLEN)
            sP2 = ps_state[0:32, 128:384].rearrange("p (h w) -> p h w", w=D)
            zr = big.tile([1, 384], BF16)
            nc.vector.memset(zr, 0.0)
            # prime the accumulator: zero-write the whole region once so all
            # chunk contributions can accumulate with start=False
            nc.tensor.matmul(ps_state[:, :], lhsT=zr[:, 0:D], rhs=zr,
                             start=True, stop=False, skip_group_check=True)

            def head_slices(h):
                g, half = h // 2, h % 2
                return g, half, slice(64 * half, 64 * half + 64)

            for i in range(NCH + 5):
                a, b, c2, pj = i, i - 1, i - 3, i - 4

                # --- evac proj(e = i-5) psum -> out_sb; DMA per 4 chunks ---
                e = i - 5
                if 0 <= e < NCH:
                    fpe = fp_t[e]
                    osb = osb_t[e // 4]
                    nc.vector.tensor_copy(osb[:, e % 4, 0:512], fpe[:, 0, :])
                    nc.scalar.copy(osb[:, e % 4, 512:1024], fpe[:, 1, :])
                    fp_t[e] = None
                    if e % 4 == 3:
                        gidx = e // 4
                        dst = out_d.rearrange("(g c p) e -> g p c e",
                                              c=4, p=128)[gidx]
                        nc.sync.dma_start(out=dst, in_=osb)

                # --- PE: m2(c2) ---
                if 0 <= c2 < NCH:
                    tok2 = slice(c2 * C, (c2 + 1) * C)
                    m2_ps = psp.tile([128, NH, C], F32, tag="pB", name="m2")
                    for h in range(NH):
                        nc.tensor.matmul(m2_ps[:, h, :],
                                         lhsT=z_strip(h)[:, tok2],
                                         rhs=awT2[:, c2, h, :],
                                         start=True, stop=True)
                    m2m = work.tile([128, NH, C], BF16, tag="m2m")
                    nc.vector.tensor_mul(m2m, m2_ps, _bcast(mask, NH))
                    m2m_t[c2] = m2m

                # --- PE: proj(pj); evac deferred to next iteration ---
                if 0 <= pj < NCH:
                    attnT = attnT_t[pj]
                    fp = psp.tile([128, 2, 512], F32, tag="pC",
                                  name="proj")
                    for nh in range(2):
                        osl = slice(nh * 512, (nh + 1) * 512)
                        for kt in range(2):
                            nc.tensor.matmul(fp[:, nh, :], lhsT=attnT[:, kt, :],
                                             rhs=wo[:, kt, osl],
                                             start=(kt == 0), stop=(kt == 1))
                    attnT_t[pj] = None
                    fp_t[pj] = fp

                # --- PE: ztp(a), m1(a) ---
                if a < NCH:
                    toka = slice(a * C, (a + 1) * C)
                    ztp = psp.tile([128, 128], F32, tag="pF", name="ztp")
                    nc.tensor.matmul(ztp, lhsT=z_cm[:, toka], rhs=id128,
                                     start=True, stop=True)
                    nc.vector.tensor_copy(z_tok[:, a, :], ztp)

                    m1_ps = psp.tile([128, NH, C], F32, tag="pA", name="m1")
                    for h in range(NH):
                        nc.tensor.matmul(m1_ps[:, h, :],
                                         lhsT=kv_at0(h)[:, toka],
                                         rhs=q_at0(h)[:, toka],
                                         start=True, stop=True)
                    m1m = work.tile([128, NH, C], BF16, tag="m1m")
                    nc.vector.tensor_mul(m1m, m1_ps, _bcast(mask, NH))
                    m1m_t[a] = m1m

                # --- PE: o1(b) + ACT exp + V softmax ---
                if 0 <= b < NCH:
                    tokb = slice(b * C, (b + 1) * C)
                    o1 = psp.tile([128, NH, PLEN], F32, tag="pD", name="o1")
                    m1mb = m1m_t[b]
                    for h in range(NH):
                        nc.tensor.matmul(o1[:, h, :], lhsT=m1mb[:, h, :],
                                         rhs=z_tok[:, b, 32 * h:32 * h + 32],
                                         start=True, stop=(b == 0))
                        if b > 0:
                            nc.tensor.matmul(o1[:, h, :],
                                             lhsT=q_at0(h)[:, tokb],
                                             rhs=P1b[:, b, h, :],
                                             start=False, stop=True)
                    m1m_t[b] = None
                    rl = smallp[:, SP_RLEN + b:SP_RLEN + b + 1]
                    e_sb = work.tile([128, NH, PLEN], F32, tag="e_sb")
                    nc.scalar.activation(out=e_sb, in_=o1, func=AF.Exp,
                                         scale=rl, bias=0.0)
                    ssum = work.tile([128, NH], F32, tag="ssum")
                    nc.vector.reduce_sum(ssum, e_sb, axis=AX.X)
                    rs = work.tile([128, NH], F32, tag="rs")
                    nc.vector.reciprocal(rs, ssum)
                    rs2 = work.tile([128, NH], F32, tag="rs2")
                    nc.vector.tensor_scalar_mul(rs2, rs, rl)
                    aw = work.tile([128, NH, PLEN], BF16, tag="aw")
                    nc.vector.tensor_mul(aw, e_sb, _bcast(rs2, PLEN, at=2))
                    aw_t[b] = aw

                # --- PE: dS1(a) dS2(a); V: prefix snapshots + adds ---
                if a < NCH:
                    if a > 0:
                        nc.vector.tensor_copy(P1b[:, a], sP1)
                        nc.vector.tensor_copy(P2b[:, a], sP2)
                    for h in range(NH):
                        g, half, _ = head_slices(h)
                        nc.tensor.matmul(sP1[:, h, :],
                                         lhsT=kvtok[:, a, g, half, :],
                                         rhs=z_tok[:, a, 32 * h:32 * h + 32],
                                         start=False, stop=False,
                                         skip_group_check=True)
                    for h in range(NH):
                        g, half, _ = head_slices(h)
                        nc.tensor.matmul(sP2[:, h, :],
                                         lhsT=z_tok[:, a, 32 * h:32 * h + 32],
                                         rhs=kvtok[:, a, g, half, :],
                                         start=False,
                                         stop=(a == NCH - 1 and h == NH - 1),
                                         skip_group_check=True)

                # --- PE: out2(c2); ACT: attnT evac ---
                if 0 <= c2 < NCH:
                    attn_ps = psp.tile([128, 2, C], F32, tag="pE",
                                       name="attn")
                    m2mc = m2m_t[c2]
                    for h in range(NH):
                        g, half, s = head_slices(h)
                        nc.tensor.matmul(attn_ps[s, g, :],
                                         lhsT=kvtok[:, c2, g, half, :],
                                         rhs=m2mc[:, h, :],
                                         start=True, stop=(c2 == 0))
                        if c2 > 0:
                            nc.tensor.matmul(attn_ps[s, g, :],
                                             lhsT=P2b[:, c2, h, :],
                                             rhs=awT2[:, c2, h, :],
                                             start=False, stop=True,
                                             tile_position=(0, 64 * (h % 2)))
                    m2m_t[c2] = None
                    attnT = work.tile([128, 2, C], BF16, tag="attnT")
                    nc.scalar.copy(attnT, attn_ps)
                    attnT_t[c2] = attnT

                # --- PE: awT(b) strip transposes; ACT: evac ---
                if 0 <= b < NCH:
                    awp = psp.tile([32, NH, C], F32, tag="pF", name="awp")
                    awb = aw_t[b]
                    for h in range(NH):
                        nc.tensor.matmul(awp[:, h, :], lhsT=awb[:, h, :],
                                         rhs=id128, start=True, stop=True)
                    aw_t[b] = None
                    nc.scalar.copy(awT2[:, b], awp)

                # allocate out_sb tile at group start
                if a < NCH and a % 4 == 0:
                    osb_t[a // 4] = outp.tile([128, 4, EMBED], BF16,
                                              tag="osb", name="osb")

    nc.compile()
    return nc


_NC = None


def get_nc():
    global _NC
    if _NC is None:
        _NC = build_nc()
    return _NC


def make_in_maps(query, pquery, Wpq, bpq, Wq, bq, Wpc, bpc, Wc, bc, Wo, bo):
    query = np.asarray(query, np.float32)
    pquery = np.asarray(pquery, np.float32)
    Wpq, Wq, Wpc, Wc, Wo = (np.asarray(w, np.float32)
                            for w in (Wpq, Wq, Wpc, Wc, Wo))
    bpq_, bq_, bpc_, bc_ = (np.asarray(v, np.float32)
                            for v in (bpq, bq, bpc, bc))
    n_idx = np.arange(NTOK, dtype=np.float64)
    rlen = (1.0 / ((n_idx + 1.0) * BETA)).astype(np.float32)
    rlen = np.ascontiguousarray(rlen.reshape(NCH, C).T)          # [C, NCH]
    mask = np.triu(np.ones((C, C), np.float32))                  # keep j <= i
    id128 = np.eye(128, dtype=np.float32)
    bf = ml_dtypes.bfloat16

    in_maps = []
    for core in range(8):
        b, hb = core // 4, core % 4
        ch = slice(hb * NH * D, (hb + 1) * NH * D)
        wqcT = np.concatenate([SCALING * Wq[ch], Wc[ch]], axis=0).T
        bqc = np.concatenate([SCALING * bq_[ch], bc_[ch]])       # [512]
        bpqs = SCALING * bpq_[ch]                                # [256]
        bpcs = bpc_[ch]                                          # [256]
        # wpcR[d, h, k, e] = Wpc_core[h*64+d, 128k+e]
        wpcR = np.ascontiguousarray(
            Wpc[ch].reshape(NH, D, 8, 128).transpose(1, 0, 2, 3))
        smallp = np.zeros((128, 64), np.float32)
        smallp[:, SP_BQC:SP_BQC + 4] = bqc.reshape(4, 128).T
        smallp[:, SP_BPQ:SP_BPQ + 2] = bpqs.reshape(2, 128).T
        # bpc0[d, h] = bpc_core[h*64+d] on partitions 0-63
        smallp[0:D, SP_BPC:SP_BPC + 4] = bpcs.reshape(NH, D).T
        smallp[:, SP_RLEN:SP_RLEN + NCH] = rlen
        in_maps.append({
            "xT": np.ascontiguousarray(query[:, b, :].T).astype(bf),
            "pxT": np.ascontiguousarray(pquery[:, b, :].T).astype(bf),
            "wqcT": np.ascontiguousarray(wqcT).astype(bf),
            "wpqT": np.ascontiguousarray((SCALING * Wpq[ch]).T).astype(bf),
            "wpcR": wpcR.astype(bf),
            "woT": np.ascontiguousarray(Wo[:, ch].T).astype(bf),
            "smallp": smallp,
            "mask": mask,
            "id128": id128.astype(bf),
        })
    return in_maps


def kernel(**inputs):
    from concourse.bass_utils import run_bass_kernel_spmd
    nc = get_nc()
    in_maps = make_in_maps(**inputs)
    res = run_bass_kernel_spmd(nc, in_maps, core_ids=list(range(8)))
    bo = np.asarray(inputs["bo"], np.float32)
    out = np.zeros((NTOK, BSZ, EMBED), np.float32)
    for b in range(BSZ):
        acc = res.results[4 * b]["out"].astype(np.float32)
        for i in range(1, 4):
            acc = acc + res.results[4 * b + i]["out"].astype(np.float32)
        out[:, b, :] = acc + bo
    return out
